# revision 16
# baseline (speedup 1.0000x reference)
"""LayerNorm-GRU Trainium2 kernel, v3.

B=64, T=512, D=256, H=512. Data-parallel over batch: 8 rows/core x 8 cores.

v3 key change vs v2: the h-side LayerNorm mean is folded into the weights.
LN(W^T h) needs no runtime mean when W's columns are pre-centered per LN
group (mean_f(W~^T h) = (rowmean_f W~) . h = 0), so the per-step stats
chain collapses to rsqrt(mean(z^2) + eps):
  - no folded-weight-sum matmuls, no mean/mis ops in the chain or apply
  - eps rides into the cross-partition ones-matmul as a second
    accumulating matmul against a constant eps column
  - quake-rsqrt (seed + 1 Newton) stays on DVE (no ACT sqrt table swap;
    the sigmoid/tanh/square/copy table set never reloads)
  - ACT per step: sigmoid + tanh only; squares moved to DVE reading PSUM
  - h_out (fp32 hist write) moved to the idle Pool engine
  - LN biases (x and h side) both folded into the phase-1 output
  - PE emission reordered (ru MMs, half the c MMs, ru stats MMs, rest of
    c MMs, ru broadcast, c stats MMs, c broadcast) so stats matmuls don't
    queue behind all c matmuls
  - hist flush (PE transposes + DMA out) happens mid-block at s==4 of the
    NEXT block, filling PE idle time instead of delaying the next step
Phase 1 (x-side projections, fp32r matmuls + LN + PE transpose to
feature-major zx in DRAM) is unchanged from v2.
"""

import os
import sys

for _p in ("/opt/trn_rl_repo", "/root/.axon_site/_ro/trn_rl_repo"):
    if os.path.isdir(_p) and _p not in sys.path:
        sys.path.insert(0, _p)

import numpy as np
import ml_dtypes
from contextlib import ExitStack

import concourse.bass as bass
import concourse.mybir as mybir
import concourse.tile as tile
from concourse import bacc
from concourse.bass import ds
from concourse.bass_utils import run_bass_kernel_spmd

F32 = mybir.dt.float32
F32R = mybir.dt.float32r
BF16 = mybir.dt.bfloat16
I32 = mybir.dt.int32
AX = mybir.AxisListType
OP = mybir.AluOpType
AF = mybir.ActivationFunctionType

B, T, D, H = 64, 512, 256, 512
NCORES = 8
BL = B // NCORES          # 8 batch rows per core
H3 = 3 * H                # 1536
NT = H3 // 128            # 12 feature tiles
NRU = (2 * H) // 128      # 8 tiles in the r|u LN group
NC_ = H // 128            # 4 tiles in the c LN group
KH = H // 128             # 4 contraction chunks for the h-matmul
ROWS = T * BL             # 4096 rows (t-major: row = t*BL + b)
EPS = 1e-5

STEPS_PER_BODY = 128
BLK = 16                  # hist flush granularity
CHUNK = 64                # steps per xfeed chunk

MAGIC = 0x5F3759DF        # quake rsqrt seed constant
NEWTON_ITERS = 1


def _build_program(general_ln: bool, sim_steps=None, newton_iters=NEWTON_ITERS):
    nc = bacc.Bacc("TRN2", target_bir_lowering=False, debug=False)

    xT_d = nc.dram_tensor("xT", [D, ROWS], F32R, kind="ExternalInput")
    wx_d = nc.dram_tensor("wx", [D, H3], F32R, kind="ExternalInput")
    whb_d = nc.dram_tensor("whb", [H, H3], BF16, kind="ExternalInput")
    h0t_d = nc.dram_tensor("h0t", [128, KH * BL], F32, kind="ExternalInput")
    ident_d = nc.dram_tensor("ident", [128, 128], F32, kind="ExternalInput")
    if general_ln:
        gx_d = nc.dram_tensor("gx", [128, H3], F32, kind="ExternalInput")
        bx_d = nc.dram_tensor("bx", [128, H3], F32, kind="ExternalInput")
        gh_d = nc.dram_tensor("gh", [128, NT], F32, kind="ExternalInput")
        bhc_d = nc.dram_tensor("bhc", [128, NC_], F32, kind="ExternalInput")
    out_d = nc.dram_tensor("out", [BL, T, H], F32, kind="ExternalOutput")
    zx_d = nc.dram_tensor("zx", [NT, 128, ROWS + CHUNK * BL], F32,
                          kind="Internal")

    with tile.TileContext(nc) as tc, ExitStack() as ctx:
        const_pool = ctx.enter_context(tc.tile_pool(name="consts", bufs=1))
        whs = const_pool.tile([128, KH, H3], BF16)
        identity = const_pool.tile([128, 128], F32)
        epsc = const_pool.tile([128, 1], F32)
        h0t = const_pool.tile([128, KH, BL], F32)
        ones8 = const_pool.tile([128, BL], F32)
        ones1 = const_pool.tile([1, 128], F32)
        onescl = const_pool.tile([128, 2], F32)   # 1/N per LN group
        epscol = const_pool.tile([128, 1], F32)   # EPS/128 for the eps-MM
        nc.vector.memset(ones8[:], 1.0)
        nc.vector.memset(ones1[:], 1.0)
        nc.vector.memset(onescl[:, 0:1], 1.0 / (2 * H))
        nc.vector.memset(onescl[:, 1:2], 1.0 / H)
        nc.vector.memset(epscol[:], EPS / 128.0)
        if general_ln:
            gx = const_pool.tile([128, H3], F32)
            bx = const_pool.tile([128, H3], F32)
            gh = const_pool.tile([128, NT], F32)
            bhc = const_pool.tile([128, NC_], F32)

        nc.sync.dma_start(whs[:], whb_d[:].rearrange("(k p) n -> p k n", p=128))
        nc.sync.dma_start(identity[:], ident_d[:])
        nc.sync.dma_start(h0t[:], h0t_d[:].rearrange("p (k b) -> p k b", k=KH))
        nc.vector.memset(epsc[:], EPS)
        if general_ln:
            nc.sync.dma_start(gx[:], gx_d[:])
            nc.sync.dma_start(bx[:], bx_d[:])
            nc.sync.dma_start(gh[:], gh_d[:])
            nc.sync.dma_start(bhc[:], bhc_d[:])

        # ================= Phase 1: x-side projections =================
        with tc.tile_pool(name="p1sbuf", bufs=1) as p1pool, \
             tc.tile_pool(name="p1work", bufs=3) as p1work, \
             tc.tile_pool(name="p1z", bufs=2, space="PSUM") as p1z, \
             tc.tile_pool(name="p1t", bufs=2, space="PSUM") as p1t:
            xts = p1pool.tile([128, 2, ROWS], F32R)
            wxs = p1pool.tile([128, 2, H3], F32R)
            nc.sync.dma_start(xts[:], xT_d[:].rearrange("(k p) n -> p k n", p=128))
            nc.sync.dma_start(wxs[:], wx_d[:].rearrange("(k p) n -> p k n", p=128))

            for r in range(ROWS // 128):
                zp = p1z.tile([128, H3], F32, tag="zp")
                for k in range(2):
                    for nb in range(3):
                        nc.tensor.matmul(
                            zp[:, nb * 512:(nb + 1) * 512],
                            xts[:, k, r * 128:(r + 1) * 128],
                            wxs[:, k, nb * 512:(nb + 1) * 512],
                            start=(k == 0), stop=(k == 1),
                        )
                sixes = p1work.tile([128, 3, 6], F32, tag="sixes")
                aggr = p1work.tile([128, 2, 2], F32, tag="aggr")
                nc.vector.bn_stats(sixes[:, 0, :], zp[:, 0:512])
                nc.vector.bn_stats(sixes[:, 1, :], zp[:, 512:1024])
                nc.vector.bn_stats(sixes[:, 2, :], zp[:, 1024:1536])
                nc.vector.bn_aggr(aggr[:, 0, :], sixes[:, 0:2, :])
                nc.vector.bn_aggr(aggr[:, 1, :], sixes[:, 2, :])
                sd = p1work.tile([128, 2], F32, tag="sd")
                inv = p1work.tile([128, 2], F32, tag="inv")
                nc.scalar.activation(sd[:], aggr[:, :, 1], AF.Sqrt, bias=epsc[:])
                nc.vector.reciprocal(inv[:], sd[:])
                zln = p1work.tile([128, H3], F32, tag="zln")
                nc.vector.tensor_scalar(
                    zln[:, 0:1024], zp[:, 0:1024],
                    aggr[:, 0, 0:1], inv[:, 0:1], OP.subtract, OP.mult)
                nc.vector.tensor_scalar(
                    zln[:, 1024:1536], zp[:, 1024:1536],
                    aggr[:, 1, 0:1], inv[:, 1:2], OP.subtract, OP.mult)
                if general_ln:
                    nc.vector.tensor_mul(zln[:], zln[:], gx[:])
                    nc.vector.tensor_add(zln[:], zln[:], bx[:])
                if r % 2 == 0:
                    ztp = p1work.tile([128, NT, 2, 128], F32, tag="ztp")
                for m in range(NT):
                    tp = p1t.tile([128, 128], F32, tag="tp")
                    nc.tensor.transpose(tp[:], zln[:, m * 128:(m + 1) * 128],
                                        identity[:])
                    # DVE is the phase-1 bottleneck (bn_stats + LN apply);
                    # route most PSUM->SBUF staging copies to ACT instead.
                    if m % 4 == 3:
                        nc.vector.tensor_copy(ztp[:, m, r % 2, :], tp[:])
                    else:
                        nc.scalar.copy(ztp[:, m, r % 2, :], tp[:])
                if r % 2 == 1:
                    nc.sync.dma_start(
                        zx_d[:, :, (r - 1) * 128:(r + 1) * 128]
                        .transpose([1, 0, 2]),
                        ztp[:].rearrange("p t two n -> p t (two n)"))

        # ================= Phase 2: recurrence =================
        xfA = const_pool.tile([128, NT, CHUNK * BL], F32)
        xfB = const_pool.tile([128, NT, CHUNK * BL], F32)
        histP = const_pool.tile([128, KH, BLK, BL], F32)
        histQ = const_pool.tile([128, KH, BLK, BL], F32)
        obuf = const_pool.tile([128, KH, 128], F32)

        nc.vector.tensor_copy(histQ[:, :, BLK - 1, :], h0t[:])
        nc.sync.dma_start(
            xfA[:], zx_d[:, :, 0:CHUNK * BL].transpose([1, 0, 2]))

        zpool = ctx.enter_context(tc.tile_pool(name="zp2", bufs=2, space="PSUM"))
        spool = ctx.enter_context(tc.tile_pool(name="sp2", bufs=2, space="PSUM"))
        tpool = ctx.enter_context(tc.tile_pool(name="tp2", bufs=1, space="PSUM"))
        wpool = ctx.enter_context(tc.tile_pool(name="w2", bufs=3))
        hpool = ctx.enter_context(tc.tile_pool(name="hb2", bufs=3))

        def quake_chain(g, v_ap, st):
            """st[0:1,0:W] = rsqrt(v_ap) via quake seed + Newton.
            v_ap: [1, W] PSUM AP holding mean(z~^2)+eps."""
            W = st.shape[1]
            nt_ = wpool.tile([1, W], I32, tag=f"nt{g}")
            nc.vector.tensor_scalar(nt_[:], v_ap.bitcast(I32), 1, -1,
                                    OP.logical_shift_right, OP.bitwise_xor)
            y_t = wpool.tile([1, W], F32, tag=f"y{g}")
            y = y_t[:]
            nc.vector.tensor_scalar(y.bitcast(I32), nt_[:], MAGIC + 1, None,
                                    OP.add)
            for it in range(newton_iters):
                a = wpool.tile([1, W], F32, tag=f"qa{g}_{it}")
                nc.vector.tensor_tensor(a[:], y, y, OP.mult)
                f_ = wpool.tile([1, W], F32, tag=f"qf{g}_{it}")
                nc.vector.scalar_tensor_tensor(f_[:], a[:], -0.5, v_ap,
                                               OP.mult, OP.mult)
                y2 = (st[0:1, 0:W] if it == newton_iters - 1 else None)
                if y2 is None:
                    y2_t = wpool.tile([1, W], F32, tag=f"qy{g}_{it}")
                    y2 = y2_t[:]
                nc.vector.scalar_tensor_tensor(y2, f_[:], 1.5, y,
                                               OP.add, OP.mult)
                y = y2

        def emit_step(h_prev, h_out, hb_prev, xf, cstep, flush=None):
            """One GRU step. h_prev/h_out: [128, KH, BL] APs (feature-major).
            hb_prev: [128, KH, BL] bf16 tile; returns the next hb tile.
            flush: optional (hist, tb_expr) to flush mid-step on PE idle."""
            z = zpool.tile([128, NT, BL], F32, tag="z")
            vps = spool.tile([1, 2 * BL], F32, tag="vps")

            # --- PE: all 48 h-matmuls ---
            for m in range(NT):
                for k in range(KH):
                    nc.tensor.matmul(
                        z[:, m, :], whs[:, k, m * 128:(m + 1) * 128],
                        hb_prev[:, k, :], start=(k == 0), stop=(k == KH - 1))
            # --- ACT: one square over both groups (PSUM -> SBUF) ---
            sq = wpool.tile([128, NT, BL], F32, tag="sq")
            nc.scalar.activation(
                sq[:].rearrange("p t b -> p (t b)"),
                z[:].rearrange("p t b -> p (t b)"), AF.Square)
            # --- DVE: per-group per-partition reduce ---
            psq_ru = wpool.tile([128, BL], F32, tag="psqru")
            nc.vector.tensor_reduce(
                psq_ru[:], sq[:, 0:NRU, :].rearrange("p t b -> p b t"),
                AX.X, OP.add)
            psq_c = wpool.tile([128, BL], F32, tag="psqc")
            nc.vector.tensor_reduce(
                psq_c[:], sq[:, NRU:NT, :].rearrange("p t b -> p b t"),
                AX.X, OP.add)
            # --- PE: cross-partition mean+eps for both groups ---
            nc.tensor.matmul(vps[0:1, 0:BL], onescl[:, 0:1], psq_ru[:],
                             start=True, stop=False)
            nc.tensor.matmul(vps[0:1, 0:BL], epscol[:], ones8[:],
                             start=False, stop=True)
            nc.tensor.matmul(vps[0:1, BL:2 * BL], onescl[:, 1:2], psq_c[:],
                             start=True, stop=False)
            nc.tensor.matmul(vps[0:1, BL:2 * BL], epscol[:], ones8[:],
                             start=False, stop=True)

            # --- DVE: one quake chain for both groups [1, 16] ---
            st = wpool.tile([1, 2 * BL], F32, tag="st")
            quake_chain("rc", vps[0:1, :], st)
            # --- Pool: broadcast inv-sigmas to all partitions (SBUF) ---
            sb = wpool.tile([128, 2, BL], F32, tag="sb")
            nc.gpsimd.partition_broadcast(
                sb[:].rearrange("p g b -> p (g b)"), st[0:1, :], channels=128)

            xs = xf[:, :, cstep * BL:(cstep + 1) * BL]
            # --- DVE: r-half apply first (it gates rh -> tanh), then
            # u-half; ACT sigmoid split to release sig_r early ---
            tru = wpool.tile([128, NRU, BL], F32, tag="tru")
            pre = wpool.tile([128, NRU, BL], F32, tag="pre")
            sig = wpool.tile([128, NRU, BL], F32, tag="sig")
            for lo, hi in ((0, NC_), (NC_, NRU)):
                nc.vector.tensor_tensor(
                    tru[:, lo:hi, :], z[:, lo:hi, :],
                    sb[:, 0:1, :].to_broadcast([128, hi - lo, BL]), OP.mult)
                if general_ln:
                    nc.vector.tensor_mul(
                        tru[:, lo:hi, :], tru[:, lo:hi, :],
                        gh[:, lo:hi].unsqueeze(2).to_broadcast(
                            [128, hi - lo, BL]))
                nc.vector.tensor_tensor(pre[:, lo:hi, :], tru[:, lo:hi, :],
                                        xs[:, lo:hi, :], OP.add)
                nc.scalar.activation(
                    sig[:, lo:hi, :].rearrange("p a b -> p (a b)"),
                    pre[:, lo:hi, :].rearrange("p a b -> p (a b)"),
                    AF.Sigmoid)

            # --- PE: mid-step flush of the previous block's hist ---
            if flush is not None:
                flush_block(*flush)

            # --- DVE: c apply; ACT tanh ---
            oc = wpool.tile([128, NC_, BL], F32, tag="oc")
            nc.vector.tensor_tensor(
                oc[:], z[:, NRU:NT, :],
                sb[:, 1:2, :].to_broadcast([128, NC_, BL]), OP.mult)
            if general_ln:
                # c-gate bias cannot fold into xs (r multiplies it):
                # oc = LN_c * g_c + b_c, then rh = r * oc.
                nc.vector.tensor_mul(
                    oc[:], oc[:],
                    gh[:, NRU:NT].unsqueeze(2).to_broadcast([128, NC_, BL]))
                nc.vector.tensor_add(
                    oc[:], oc[:],
                    bhc[:].unsqueeze(2).to_broadcast([128, NC_, BL]))
            rh = wpool.tile([128, NC_, BL], F32, tag="rh")
            nc.vector.tensor_tensor(rh[:], sig[:, 0:NC_, :], oc[:], OP.mult)
            prec = wpool.tile([128, NC_, BL], F32, tag="prec")
            nc.vector.tensor_tensor(prec[:], rh[:], xs[:, NRU:NT, :], OP.add)
            cc = wpool.tile([128, NC_, BL], F32, tag="cc")
            nc.scalar.activation(
                cc[:].rearrange("p a b -> p (a b)"),
                prec[:].rearrange("p a b -> p (a b)"), AF.Tanh)

            # --- DVE: gate tail. h_new = (h - u*h) + u*c; u*h and the
            # subtraction run during tanh so only 2 ops follow it. ---
            uh = wpool.tile([128, KH, BL], F32, tag="uh")
            nc.vector.tensor_tensor(uh[:], sig[:, NC_:NRU, :], h_prev,
                                    OP.mult)
            t1 = wpool.tile([128, KH, BL], F32, tag="t1")
            nc.vector.tensor_tensor(t1[:], h_prev, uh[:], OP.subtract)
            uc = wpool.tile([128, KH, BL], F32, tag="uc")
            nc.vector.tensor_tensor(uc[:], sig[:, NC_:NRU, :], cc[:],
                                    OP.mult)
            hb = hpool.tile([128, KH, BL], BF16, tag="hb")
            nc.vector.tensor_tensor(hb[:], t1[:], uc[:], OP.add)
            # fp32 hist/output copy on the idle Pool engine, off the
            # critical path
            nc.gpsimd.tensor_tensor(h_out, t1[:], uc[:], OP.add)
            return hb

        def flush_block(hist, tb_expr):
            for k in range(KH):
                tp = tpool.tile([128, 128], F32, tag="ftp")
                nc.tensor.transpose(tp[:], hist[:, k, :, :], identity[:])
                if k % 2 == 0:
                    nc.scalar.copy(obuf[:, k, :], tp[:])
                else:
                    nc.vector.tensor_copy(obuf[:, k, :], tp[:])
            nc.sync.dma_start(
                out_d[:, ds(tb_expr, BLK), :].transpose([1, 0, 2]),
                obuf[:].rearrange("p k n -> p (k n)"))

        def _emit_body(ib):
            hb = hpool.tile([128, KH, BL], BF16, tag="hb")
            nc.vector.tensor_copy(hb[:], histQ[:, :, BLK - 1, :])
            nc.sync.dma_start(
                xfB[:],
                zx_d[:, :, ds((ib + CHUNK) * BL, CHUNK * BL)].transpose([1, 0, 2]))
            for half in range(2):
                xf = (xfA, xfB)[half]
                for blk in range(4):
                    gblk = half * 4 + blk
                    hist = (histP, histQ)[gblk % 2]
                    prev_hist = (histP, histQ)[(gblk + 1) % 2]
                    for s in range(BLK):
                        cstep = blk * BLK + s
                        h_prev = (hist[:, :, s - 1, :] if s > 0
                                  else prev_hist[:, :, BLK - 1, :])
                        flush = None
                        if s == 4 and gblk > 0:
                            flush = (prev_hist, ib + (gblk - 1) * BLK)
                        hb = emit_step(h_prev, hist[:, :, s, :], hb, xf,
                                       cstep, flush=flush)
                    if gblk == 7:
                        flush_block(hist, ib + 7 * BLK)
            nc.sync.dma_start(
                xfA[:],
                zx_d[:, :, ds((ib + 2 * CHUNK) * BL, CHUNK * BL)].transpose([1, 0, 2]))

        if sim_steps is not None:
            for ib2 in range(0, sim_steps, STEPS_PER_BODY):
                _emit_body(ib2)
        else:
            with tc.For_i(0, T, STEPS_PER_BODY,
                          hint_engines=(mybir.EngineType.PE,
                                        mybir.EngineType.DVE,
                                        mybir.EngineType.Activation,
                                        mybir.EngineType.Pool)) as ib:
                _emit_body(ib)

    nc.compile()
    return nc


_CACHE = {}
LAST_RESULT = None


def _get_program(general_ln: bool):
    if general_ln not in _CACHE:
        _CACHE[general_ln] = _build_program(general_ln)
    return _CACHE[general_ln]


def build_in_maps(inputs):
    return _prep(**inputs)[0]


def _prep(x, W_xr, W_xu, W_xc, W_hr, W_hu, W_hc, h0,
          ln_xru_scale, ln_xru_bias, ln_hru_scale, ln_hru_bias,
          ln_xc_scale, ln_xc_bias, ln_hc_scale, ln_hc_bias):
    x = np.ascontiguousarray(np.asarray(x, np.float32))
    wx = np.concatenate([W_xr, W_xu, W_xc], axis=1).astype(np.float32)
    wh = np.concatenate([W_hr, W_hu, W_hc], axis=1).astype(np.float32)
    # Pre-center each LN group's columns so mean_f(W~^T h) == 0: the
    # runtime LN mean vanishes and only sum(z^2) is needed per step.
    wh[:, :2 * H] -= wh[:, :2 * H].mean(axis=1, keepdims=True)
    wh[:, 2 * H:] -= wh[:, 2 * H:].mean(axis=1, keepdims=True)
    whb = np.ascontiguousarray(wh.astype(ml_dtypes.bfloat16))

    gx_full = np.concatenate([ln_xru_scale, ln_xc_scale]).astype(np.float32)
    bx_full = np.concatenate([ln_xru_bias, ln_xc_bias]).astype(np.float32)
    gh_full = np.concatenate([ln_hru_scale, ln_hc_scale]).astype(np.float32)
    bh_full = np.concatenate([ln_hru_bias, ln_hc_bias]).astype(np.float32)
    general_ln = not (np.all(gx_full == 1) and np.all(bx_full == 0)
                      and np.all(gh_full == 1) and np.all(bh_full == 0))

    h0 = np.asarray(h0, np.float32)
    h0t = np.repeat(h0.reshape(KH, 128).T[:, :, None], BL, axis=2)
    h0t = np.ascontiguousarray(h0t.reshape(128, KH * BL), np.float32)

    ident = np.eye(128, dtype=np.float32)

    shared = {
        "wx": np.ascontiguousarray(wx), "whb": whb,
        "h0t": h0t, "ident": ident,
    }
    if general_ln:
        # Fold the r|u h-side LN bias into the phase-1 output (phase 2
        # adds xs which then carries bx + bh_ru). The c-gate bias stays
        # separate: reference multiplies it by r before adding xc.
        bfold = bx_full.copy()
        bfold[:2 * H] += bh_full[:2 * H]
        shared["gx"] = np.broadcast_to(gx_full, (128, H3)).copy()
        shared["bx"] = np.broadcast_to(bfold, (128, H3)).copy()
        shared["gh"] = np.ascontiguousarray(gh_full.reshape(NT, 128).T)
        shared["bhc"] = np.ascontiguousarray(
            bh_full[2 * H:].reshape(NC_, 128).T)

    in_maps = []
    for c in range(NCORES):
        xl = x[c * BL:(c + 1) * BL]                      # [BL, T, D]
        xT = np.ascontiguousarray(
            xl.transpose(2, 1, 0).reshape(D, ROWS), np.float32)
        in_maps.append({"xT": xT, **shared})

    return in_maps, general_ln


def kernel(**inputs):
    in_maps, general_ln = _prep(**inputs)
    nc = _get_program(general_ln)
    res = run_bass_kernel_spmd(nc, in_maps, list(range(NCORES)))
    global LAST_RESULT
    LAST_RESULT = res
    outs = [res.results[c]["out"] for c in range(NCORES)]
    return np.concatenate(outs, axis=0).astype(np.float32)


if __name__ == "__main__":
    rng = np.random.default_rng(0)
    ins = {
        "x": rng.standard_normal((B, T, D), dtype=np.float32),
        "W_xr": rng.standard_normal((D, H), dtype=np.float32) / np.sqrt(D),
        "W_xu": rng.standard_normal((D, H), dtype=np.float32) / np.sqrt(D),
        "W_xc": rng.standard_normal((D, H), dtype=np.float32) / np.sqrt(D),
        "W_hr": rng.standard_normal((H, H), dtype=np.float32) / np.sqrt(H),
        "W_hu": rng.standard_normal((H, H), dtype=np.float32) / np.sqrt(H),
        "W_hc": rng.standard_normal((H, H), dtype=np.float32) / np.sqrt(H),
        "h0": np.zeros(H, np.float32),
        "ln_xru_scale": np.ones(2 * H, np.float32),
        "ln_xru_bias": np.zeros(2 * H, np.float32),
        "ln_hru_scale": np.ones(2 * H, np.float32),
        "ln_hru_bias": np.zeros(2 * H, np.float32),
        "ln_xc_scale": np.ones(H, np.float32),
        "ln_xc_bias": np.zeros(H, np.float32),
        "ln_hc_scale": np.ones(H, np.float32),
        "ln_hc_bias": np.zeros(H, np.float32),
    }
    out = kernel(**ins)
    print(out.shape, out.dtype, np.abs(out).mean())
